# revision 1
# baseline (speedup 1.0000x reference)
"""ChildSum TreeLSTM cell on 8 Trainium2 NeuronCores (Bass/Tile, SPMD).

Sharding: nodes split evenly (2048/core); each core's children (contiguous,
since seg_ids is sorted) are re-laid out host-side into a window-aligned
padded layout: 16 node-windows of 128 nodes per core, each window's children
padded to KMAX slots of 128 rows.  Segment sums become per-window one-hot
matmuls accumulated in PSUM; the one-hot masks are built on device from a
shipped rel-index tensor (seg - window_base), so a single data-independent
program serves all 8 cores.

Per-core device program (all matmuls fp32r or bf16, 1 PE cycle/row):
  f_inp[w]   = x_w @ Wwf.T + (bwf+buf)              (stored bf16, per window)
  per slot s of window w (128 children):
    S_cn[p,j] = (rel[p] == j)                        (DVE iota compare)
    S_nc      = S_cn.T                               (PE transpose)
    fh+g      = prevh_slot @ Wuf.T + S_cn @ f_inp[w] (one PSUM accum group)
    f_jk      = sigmoid(fh+g)                        (ACT)
    t         = f_jk * prevc_slot                    (DVE)
    fc[w]    += S_cn.T @ t                           (PSUM accum over slots)
    htT[w]   += prevh_slot.T @ S_cn                  (PSUM accum, transposed)
  big = [x_w.T; htT].T @ Wc.T + bc ; gates; c = sig(zi)*tanh(zu)+fc ;
  h = sig(zo)*tanh(c)
"""

import numpy as np
import ml_dtypes

import concourse.bass as bass
import concourse.bacc as bacc
import concourse.mybir as mybir
from concourse import tile
from concourse.bass_utils import run_bass_kernel_spmd

BF16 = ml_dtypes.bfloat16
F32 = mybir.dt.float32
F32R = mybir.dt.float32r
BF = mybir.dt.bfloat16

N, E, D, H = 16384, 65536, 512, 512
NCORES = 8
NL = N // NCORES            # 2048 local nodes
NW = NL // 128              # 16 windows
H3 = 3 * H

AF = mybir.ActivationFunctionType
ALU = mybir.AluOpType

# ---------------------------------------------------------------------------
# Host-side shard planning and per-core data layout
# ---------------------------------------------------------------------------
def _plan(seg):
    win_edges = np.arange(0, N + 1, 128)
    wchild = np.searchsorted(seg, win_edges)
    kmax = int(np.max(np.ceil(np.diff(wchild) / 128.0)))
    return wchild, max(kmax, 1)


def _prep_core(inputs, core, wchild, kmax):
    seg = inputs["seg_ids"]
    x, prev_c, prev_h = inputs["x"], inputs["prev_c"], inputs["prev_h"]
    g0 = core * NL
    S = NW * kmax * 128

    prevh_n = np.zeros((S, H), np.float32)
    prevc_n = np.zeros((S, H), np.float32)
    rel = np.full((S,), -1.0, np.float32)
    for w in range(NW):
        gw = core * NW + w
        ws, we = int(wchild[gw]), int(wchild[gw + 1])
        base = w * kmax * 128
        prevh_n[base : base + we - ws] = prev_h[ws:we]
        prevc_n[base : base + we - ws] = prev_c[ws:we]
        rel[base : base + we - ws] = (seg[ws:we] - (g0 + 128 * w)).astype(np.float32)

    return {
        "x_T": np.ascontiguousarray(x[g0 : g0 + NL].T),                  # f32 [D, NL]
        "prevh_T": np.ascontiguousarray(prevh_n.T).astype(BF16),         # [H, S]
        "prevh_n": prevh_n.astype(BF16),                                 # [S, H]
        "prevc_n": prevc_n,                                              # f32 [S, H]
        "relcol": np.ascontiguousarray(rel.reshape(S // 128, 128).T),    # f32 [128, S/128]
    }


def _prep_shared(inputs):
    Wc, bc = inputs["Wc"], inputs["bc"]
    Wwf, bwf = inputs["Wwf"], inputs["bwf"]
    Wuf, buf = inputs["Wuf"], inputs["buf"]
    ones = np.ones((128, 1), np.float32)
    return {
        "Wwf_T": np.ascontiguousarray(Wwf.T),                            # f32 [D, H]
        "Wuf_T": np.ascontiguousarray(Wuf.T).astype(BF16),               # [H, H]
        "Wc_T": np.ascontiguousarray(Wc.T),                              # f32 [D+H, 3H]
        "B1": (ones * (bwf + buf)[None, :]).astype(np.float32),          # [128, H]
        "bc_b": (ones * bc[None, :]).astype(np.float32),                 # [128, 3H]
        "iota": np.broadcast_to(
            np.arange(128, dtype=np.float32)[None, :], (128, 128)
        ).copy(),                                                        # [128, 128]
        "ident": np.eye(128, dtype=np.float32).astype(BF16),             # [128, 128]
    }


# ---------------------------------------------------------------------------
# Device program (identical for all cores; per-core data differs)
# ---------------------------------------------------------------------------
def _build_program(kmax, repeat=1):
    """repeat>1 wraps the whole body in a hardware loop (timing harness only)."""
    SLOTS = NW * kmax
    S = SLOTS * 128

    nc = bacc.Bacc(None, target_bir_lowering=False)
    d_xT = nc.dram_tensor("x_T", [D, NL], F32R, kind="ExternalInput")
    d_phT = nc.dram_tensor("prevh_T", [H, S], BF, kind="ExternalInput")
    d_phn = nc.dram_tensor("prevh_n", [S, H], BF, kind="ExternalInput")
    d_pc = nc.dram_tensor("prevc_n", [S, H], F32, kind="ExternalInput")
    d_rel = nc.dram_tensor("relcol", [128, SLOTS], F32, kind="ExternalInput")
    d_wwf = nc.dram_tensor("Wwf_T", [D, H], F32R, kind="ExternalInput")
    d_wuf = nc.dram_tensor("Wuf_T", [H, H], BF, kind="ExternalInput")
    d_wc = nc.dram_tensor("Wc_T", [D + H, H3], F32R, kind="ExternalInput")
    d_b1 = nc.dram_tensor("B1", [128, H], F32, kind="ExternalInput")
    d_bcb = nc.dram_tensor("bc_b", [128, H3], F32, kind="ExternalInput")
    d_iota = nc.dram_tensor("iota", [128, 128], F32, kind="ExternalInput")
    d_ident = nc.dram_tensor("ident", [128, 128], BF, kind="ExternalInput")
    d_c = nc.dram_tensor("c_out", [NL, H], F32, kind="ExternalOutput")
    d_h = nc.dram_tensor("h_out", [NL, H], F32, kind="ExternalOutput")

    import contextlib

    with tile.TileContext(nc) as tc:
        with (
            tc.tile_pool(name="const", bufs=1) as cpool,
            tc.tile_pool(name="stream", bufs=2) as spool,
            tc.tile_pool(name="work", bufs=3) as wpool,
            tc.tile_pool(name="snc", bufs=2) as sncpool,
            tc.tile_pool(name="gates", bufs=2) as gpool,
            tc.tile_pool(name="pfhg", bufs=2, space="PSUM") as pfhg,
            tc.tile_pool(name="phtT", bufs=2, space="PSUM") as phtT,
            tc.tile_pool(name="pfc", bufs=2, space="PSUM") as pfc,
            tc.tile_pool(name="pbig", bufs=2, space="PSUM") as pbig,
            tc.For_i(0, repeat, 1) if repeat > 1 else contextlib.nullcontext(),
        ):
            # ---- resident constants -------------------------------------
            # Order matters: PE's first work (f_inp window 0) needs only the
            # small tensors + one x column group, so emit those first and
            # defer the big Wc load past window 0's streams.
            iota = cpool.tile([128, 128], F32)
            nc.sync.dma_start(iota[:], d_iota[:])
            ident = cpool.tile([128, 128], BF)
            nc.sync.dma_start(ident[:], d_ident[:])
            relc = cpool.tile([128, SLOTS], F32)
            nc.sync.dma_start(relc[:], d_rel[:])
            b1 = cpool.tile([128, H], F32)
            nc.sync.dma_start(b1[:], d_b1[:])
            wwf = cpool.tile([128, 4, H], F32R)
            nc.sync.dma_start(wwf[:], d_wwf.rearrange("(q p) h -> p q h", p=128))
            wuf = cpool.tile([128, 4, H], BF)
            nc.sync.dma_start(wuf[:], d_wuf.rearrange("(q p) h -> p q h", p=128))
            xT = cpool.tile([128, 4, NL], F32R)
            xT_r = d_xT.rearrange("(q p) j -> p q j", p=128)
            wc = cpool.tile([128, 8, H3], F32R)
            bcb = cpool.tile([128, H3], F32)
            finp = cpool.tile([128, NW, H], BF)

            # ---- main loop over windows ---------------------------------
            phT_r = d_phT.rearrange("(q p) (w s) -> p q w s", p=128, w=NW)
            phn_r = d_phn.rearrange("(w k p) h -> p w k h", p=128, w=NW)
            pc_r = d_pc.rearrange("(w k p) h -> p w k h", p=128, w=NW)

            for w in range(NW):
                # f_inp for this window (bf16, bias folded in) — emitted first
                # so PE can start on it as soon as its small x column group
                # lands, while the bulk window streams are still in flight.
                wsl = slice(128 * w, 128 * (w + 1))
                nc.sync.dma_start(xT[:, :, wsl], xT_r[:, :, wsl])
                fp = pfhg.tile([128, H], F32, tag="fhg")
                for q in range(4):
                    nc.tensor.matmul(
                        fp[:], xT[:, q, wsl], wwf[:, q, :],
                        start=(q == 0), stop=(q == 3),
                    )
                nc.vector.tensor_tensor(finp[:, w, :], fp[:], b1[:], op=ALU.add)

                # window streams
                phT = spool.tile([128, 4, kmax * 128], BF, tag="phT")
                nc.sync.dma_start(phT[:], phT_r[:, :, w, :])
                phn = spool.tile([128, kmax, H], BF, tag="phn")
                nc.sync.dma_start(phn[:], phn_r[:, w, :, :])
                pc = spool.tile([128, kmax, H], F32, tag="pc")
                nc.sync.dma_start(pc[:], pc_r[:, w, :, :])
                if w == 0:
                    # big Wc/bc loads deferred here: needed only at window 0's
                    # tail, ~12 us after PE starts on f_inp/slots.  Split by
                    # z-chunk so the zc=0 slice lands first.
                    wc_r = d_wc.rearrange("(q p) z -> p q z", p=128)
                    for zc in range(3):
                        zsl = slice(H * zc, H * (zc + 1))
                        nc.sync.dma_start(wc[:, :, zsl], wc_r[:, :, zsl])
                    nc.sync.dma_start(bcb[:], d_bcb[:])

                # one-hot masks + transposes
                stp = pfhg.tile([128, kmax * 128], BF, tag="fhg")
                s32t = sncpool.tile([128, kmax * 128], F32R, tag="s32")
                s16t = sncpool.tile([128, kmax * 128], BF, tag="s16")
                for k in range(kmax):
                    s = w * kmax + k
                    ksl = slice(128 * k, 128 * (k + 1))
                    nc.vector.tensor_scalar(
                        s32t[:, ksl], iota[:], relc[:, s : s + 1], None, op0=ALU.is_equal
                    )
                    nc.vector.tensor_scalar(
                        s16t[:, ksl], iota[:], relc[:, s : s + 1], None, op0=ALU.is_equal
                    )
                    nc.tensor.transpose(stp[:, ksl], s16t[:, ksl], ident[:])
                snc = sncpool.tile([128, kmax * 128], BF, tag="snc")
                nc.vector.tensor_copy(snc[:], stp[:])

                fcp = pfc.tile([128, H], F32, tag="fc")
                htp = phtT.tile([128, H], F32, tag="htT")
                for k in range(kmax):
                    fhg = pfhg.tile([128, H], F32, tag="fhg")
                    for q in range(4):
                        nc.tensor.matmul(
                            fhg[:],
                            phT[:, q, 128 * k : 128 * (k + 1)],
                            wuf[:, q, :],
                            start=(q == 0),
                            stop=False,
                        )
                    nc.tensor.matmul(
                        fhg[:],
                        snc[:, 128 * k : 128 * (k + 1)],
                        finp[:, w, :],
                        start=False,
                        stop=True,
                    )
                    fjk = wpool.tile([128, H], F32, tag="fjk")
                    nc.scalar.activation(fjk[:], fhg[:], AF.Sigmoid)
                    t = wpool.tile([128, H], F32R, tag="t")
                    nc.vector.tensor_tensor(t[:], fjk[:], pc[:, k, :], op=ALU.mult)
                    nc.tensor.matmul(
                        fcp[:],
                        s32t[:, 128 * k : 128 * (k + 1)],
                        t[:],
                        start=(k == 0),
                        stop=(k == kmax - 1),
                    )
                # h_tilde^T: q outer so each psum slice runs one accumulation
                # group at a time (interleaved slice-groups trip the zero-
                # region group check).
                for q in range(4):
                    for k in range(kmax):
                        nc.tensor.matmul(
                            htp[:, 128 * q : 128 * (q + 1)],
                            phn[:, k, 128 * q : 128 * (q + 1)],
                            s16t[:, 128 * k : 128 * (k + 1)],
                            start=(k == 0),
                            stop=(k == kmax - 1),
                        )

                # window tail: big matmul + gates
                hts = gpool.tile([128, H], F32R, tag="hts")
                nc.vector.tensor_copy(hts[:], htp[:])
                fcs = gpool.tile([128, H], F32, tag="fcs")
                nc.vector.tensor_copy(fcs[:], fcp[:])

                zt = []
                for zc in range(3):
                    bp = pbig.tile([128, H], F32, tag="big")
                    for kc in range(8):
                        if kc < 4:
                            lhsT = xT[:, kc, 128 * w : 128 * (w + 1)]
                        else:
                            lhsT = hts[:, 128 * (kc - 4) : 128 * (kc - 3)]
                        nc.tensor.matmul(
                            bp[:],
                            lhsT,
                            wc[:, kc, H * zc : H * (zc + 1)],
                            start=(kc == 0),
                            stop=(kc == 7),
                        )
                    zs = gpool.tile([128, H], F32, tag=f"z{zc}")
                    nc.vector.tensor_tensor(
                        zs[:], bp[:], bcb[:, H * zc : H * (zc + 1)], op=ALU.add
                    )
                    zt.append(zs)
                zi, zo, zu = zt
                nc.scalar.activation(zi[:], zi[:], AF.Sigmoid)
                nc.scalar.activation(zo[:], zo[:], AF.Sigmoid)
                nc.scalar.activation(zu[:], zu[:], AF.Tanh)
                ct = gpool.tile([128, H], F32, tag="ct")
                nc.vector.tensor_tensor(ct[:], zi[:], zu[:], op=ALU.mult)
                nc.vector.tensor_tensor(ct[:], ct[:], fcs[:], op=ALU.add)
                tc_t = gpool.tile([128, H], F32, tag="tct")
                nc.scalar.activation(tc_t[:], ct[:], AF.Tanh)
                nc.vector.tensor_tensor(zo[:], zo[:], tc_t[:], op=ALU.mult)
                nc.sync.dma_start(d_c[128 * w : 128 * (w + 1), :], ct[:])
                nc.sync.dma_start(d_h[128 * w : 128 * (w + 1), :], zo[:])

    nc.compile()
    return nc


# ---------------------------------------------------------------------------
# Entry point
# ---------------------------------------------------------------------------
def kernel(**inputs):
    inputs = {k: np.asarray(v) for k, v in inputs.items()}
    seg = inputs["seg_ids"]
    assert seg.shape == (E,) and np.all(np.diff(seg) >= 0)

    wchild, kmax = _plan(seg)
    shared = _prep_shared(inputs)
    in_maps = []
    for core in range(NCORES):
        m = dict(shared)
        m.update(_prep_core(inputs, core, wchild, kmax))
        in_maps.append(m)

    nc = _build_program(kmax)
    res = run_bass_kernel_spmd(nc, in_maps, list(range(NCORES)))

    c = np.concatenate([res.results[i]["c_out"] for i in range(NCORES)], axis=0)
    h = np.concatenate([res.results[i]["h_out"] for i in range(NCORES)], axis=0)
    return (c.astype(np.float32), h.astype(np.float32))

